# revision 1
# baseline (speedup 1.0000x reference)
"""Trainium2 Bass kernel for nn_CrossAttention (B=2, N=2048, M=256, C=1024, H=16).

Sharding: 8 cores = 2 batches x 4 head-groups (4 heads each).
Each core computes its heads' QKV/KV projections, qk-RMSNorm, attention and a
partial output projection over its 256 channels; the host sums the 4 partials
per batch (the all-reduce) and adds proj_b.
"""

import sys

sys.path.insert(0, "/opt/trn_rl_repo")

import numpy as np  # noqa: E402

import concourse.bass as bass  # noqa: E402
import concourse.tile as tile  # noqa: E402
from concourse import bacc, mybir  # noqa: E402
from concourse.bass_utils import run_bass_kernel_spmd  # noqa: E402

F32 = mybir.dt.float32
R32 = mybir.dt.float32r
AF = mybir.ActivationFunctionType
MUL = mybir.AluOpType.mult

H = 16
B = 2
N = 2048          # image tokens
M = 256           # text tokens
C = 1024
HD = 64           # head dim
EPS = 1e-6
S = N + M         # 2304 kv length
HPC = 4           # heads per core
NT = 512          # query tile
SCALE = HD ** -0.5





_TCNT = [0]


def T(pool, shape, tag, bufs=None, dt=F32):
    _TCNT[0] += 1
    kw = dict(tag=tag, name=f"{tag}_{_TCNT[0]}")
    if bufs is not None:
        kw["bufs"] = bufs
    return pool.tile(shape, dt, **kw)


def build_program(loop_iters=None):
    nc = bacc.Bacc("TRN2", target_bir_lowering=False, debug=False)

    xT = nc.dram_tensor("xT", [C, N], R32, kind="ExternalInput").ap()
    yT = nc.dram_tensor("yT", [C, M], R32, kind="ExternalInput").ap()
    wqkvT = nc.dram_tensor("wqkvT", [C, 2 * HPC * HD], R32, kind="ExternalInput").ap()
    bqkv = nc.dram_tensor("bqkv", [128, 4], F32, kind="ExternalInput").ap()
    wkvT = nc.dram_tensor("wkvT", [C, HPC * HD], R32, kind="ExternalInput").ap()
    wvxT = nc.dram_tensor("wvxT", [C, HPC * HD], R32, kind="ExternalInput").ap()
    wvyT = nc.dram_tensor("wvyT", [C, HPC * HD], R32, kind="ExternalInput").ap()
    bvx = nc.dram_tensor("bvx", [1, HPC * HD], R32, kind="ExternalInput").ap()
    bvy = nc.dram_tensor("bvy", [1, HPC * HD], R32, kind="ExternalInput").ap()
    ones1r = nc.dram_tensor("ones1r", [1, 128], R32, kind="ExternalInput").ap()
    bkv = nc.dram_tensor("bkv", [128, 2], F32, kind="ExternalInput").ap()
    wprojT = nc.dram_tensor("wprojT", [HPC * HD, C], R32, kind="ExternalInput").ap()
    qknw = nc.dram_tensor("qknw", [128, 2], F32, kind="ExternalInput").ap()
    onesb = nc.dram_tensor("onesb", [128, 2], R32, kind="ExternalInput").ap()
    ones2 = nc.dram_tensor("ones2", [2, 128], R32, kind="ExternalInput").ap()
    sel64 = nc.dram_tensor("sel64", [65, 64], R32, kind="ExternalInput").ap()
    vones = nc.dram_tensor("vones", [128, 18, 1], R32, kind="ExternalInput").ap()
    outT = nc.dram_tensor("outT", [C, N], F32, kind="ExternalOutput").ap()

    with tile.TileContext(nc) as tc:
        with (
            tc.tile_pool(name="const", bufs=1) as const,
            tc.tile_pool(name="sing", bufs=1) as sing,
        ):
            yT_sb = T(const, [128, 8, M], "yT", dt=R32)
            nc.sync.dma_start(yT_sb, yT.rearrange("(o p) f -> p o f", p=128))
            wkv_sb = T(const, [128, 8, HPC * HD], "wkv", dt=R32)
            nc.sync.dma_start(wkv_sb, wkvT.rearrange("(o p) f -> p o f", p=128))
            wvy_sb = T(const, [128, 8, HPC * HD], "wvy", dt=R32)
            nc.sync.dma_start(wvy_sb, wvyT.rearrange("(o p) f -> p o f", p=128))
            wvx_sb = T(const, [128, 8, HPC * HD], "wvx", dt=R32)
            nc.sync.dma_start(wvx_sb, wvxT.rearrange("(o p) f -> p o f", p=128))
            bvx_sb = T(const, [1, HPC * HD], "bvx", dt=R32)
            nc.sync.dma_start(bvx_sb, bvx)
            bvy_sb = T(const, [1, HPC * HD], "bvy", dt=R32)
            nc.sync.dma_start(bvy_sb, bvy)
            ones1_sb = T(const, [1, 128], "ones1r", dt=R32)
            nc.sync.dma_start(ones1_sb, ones1r)
            wqkv_sb = T(const, [128, 8, 2 * HPC * HD], "wqkv", dt=R32)
            wqkv_r = wqkvT.rearrange("(o p) f -> p o f", p=128)
            for cc in range(8):
                nc.sync.dma_start(wqkv_sb[:, cc], wqkv_r[:, cc])
            wproj_sb = T(const, [64, 4, C], "wproj", dt=R32)
            nc.sync.dma_start(wproj_sb, wprojT.rearrange("(c p) o -> p c o", p=64))
            bqkv_sb = T(const, [128, 4], "bqkv")
            nc.sync.dma_start(bqkv_sb, bqkv)
            bkv_sb = T(const, [128, 2], "bkv")
            nc.sync.dma_start(bkv_sb, bkv)
            qknw_sb = T(const, [128, 2], "qknw")
            nc.sync.dma_start(qknw_sb, qknw)
            onesb_sb = T(const, [128, 2], "onesb", dt=R32)
            nc.sync.dma_start(onesb_sb, onesb)
            ones2_sb = T(const, [2, 128], "ones2", dt=R32)
            nc.sync.dma_start(ones2_sb, ones2)
            sel_sb = T(const, [65, 64], "sel64", dt=R32)
            nc.sync.dma_start(sel_sb, sel64)
            eps_sb = T(const, [128, 1], "epsc")
            nc.vector.memset(eps_sb, float(EPS))
            zero_sb = T(const, [128, 1], "zeroc")
            nc.vector.memset(zero_sb, 0.0)

            # persistent activations: channel-on-partition layouts
            qT = T(sing, [128, 2, N], "qT", dt=R32)       # [2 heads x 64d, hp, n]
            kT = T(sing, [128, 2, S], "kT", dt=R32)
            vS = T(sing, [128, 18, HPC * 65], "vS", dt=R32)  # [s%128, s//128, h*65+(d|one)]
            for h in range(HPC):
                nc.sync.dma_start(vS[:, :, 65 * h + 64 : 65 * h + 65], vones)

            def norm_chunk(pool_ps, pool_wk, psum, bias_ap, w_col, dest):
                """dest = (psum + bias) * rsqrt(mean_d((psum+bias)^2)+eps) * w"""
                nsz = psum.shape[-1]
                tb = T(pool_wk, [128, NT], "w")[:, :nsz]
                nc.vector.tensor_scalar_add(tb, psum, bias_ap)
                sq = T(pool_wk, [128, NT], "w", dt=R32)[:, :nsz]
                nc.vector.tensor_mul(sq, tb, tb)
                ssp = T(pool_ps, [2, NT], "paux", bufs=3)[:, :nsz]
                nc.tensor.matmul(ssp, onesb_sb, sq, start=True, stop=True)
                lnv = T(pool_wk, [2, NT], "w2", bufs=8)[:, :nsz]
                nc.scalar.activation(
                    lnv, ssp, AF.Ln, bias=eps_sb[0:2], scale=1.0 / HD
                )
                rmsv = T(pool_wk, [2, NT], "w2", bufs=8, dt=R32)[:, :nsz]
                nc.scalar.activation(rmsv, lnv, AF.Exp, bias=zero_sb[0:2], scale=-0.5)
                rbc = T(pool_ps, [128, NT], "paux", bufs=3)[:, :nsz]
                nc.tensor.matmul(rbc, ones2_sb, rmsv, start=True, stop=True)
                t2 = T(pool_wk, [128, NT], "w")[:, :nsz]
                nc.vector.tensor_mul(t2, tb, rbc)
                nc.vector.tensor_scalar_mul(dest, t2, w_col)

            def v_proj(pool_ps, src_sb, t, w_sb, b_sb, j):
                """vS[:, j] = (src.T @ wv + bv) directly in [s, d] layout."""
                pv = T(pool_ps, [128, HPC * HD], "pmain", bufs=4)
                for cc in range(8):
                    nc.tensor.matmul(
                        pv,
                        src_sb[:, cc, t * 128 : (t + 1) * 128],
                        w_sb[:, cc, :],
                        start=(cc == 0),
                        stop=False,
                    )
                nc.tensor.matmul(pv, ones1_sb, b_sb, start=False, stop=True)
                dst = vS[:, j, :].rearrange("p (a b) -> p a b", b=65)[:, :, 0:64]
                nc.vector.tensor_copy(
                    out=dst, in_=pv.rearrange("p (a b) -> p a b", b=64)
                )

            # ---- phase 1: KV projection of y (text tokens -> kv rows 2048..2303)
            import contextlib
            with contextlib.ExitStack() as _les:
                if loop_iters is not None:
                    _les.enter_context(tc.For_i(0, loop_iters, 1))
                with (
                    tc.tile_pool(name="pp12", bufs=3, space="PSUM") as pp12,
                    tc.tile_pool(name="wk", bufs=12) as wk,
                ):
                    for mc in range(2):  # [k01, k23]
                        ps = T(pp12, [128, NT], "pmain", bufs=4)[:, :M]
                        for cc in range(8):
                            nc.tensor.matmul(
                                ps,
                                wkv_sb[:, cc, mc * 128 : (mc + 1) * 128],
                                yT_sb[:, cc, :],
                                start=(cc == 0),
                                stop=(cc == 7),
                            )
                        norm_chunk(
                            pp12, wk, ps, bkv_sb[:, mc : mc + 1],
                            qknw_sb[:, 1:2], kT[:, mc, N : N + M],
                        )
                    for t in range(2):
                        v_proj(pp12, yT_sb, t, wvy_sb, bvy_sb, 16 + t)

                    # ---- phase 2: QKV projection of x
                    with tc.tile_pool(name="xin", bufs=2) as xin:
                        for nt in range(N // NT):
                            nsl = slice(nt * NT, (nt + 1) * NT)
                            xc = T(xin, [128, 8, NT], "xc", dt=R32)
                            nc.sync.dma_start(
                                xc, xT.rearrange("(o p) f -> p o f", p=128)[:, :, nsl]
                            )
                            for mc in range(4):  # [q01,q23,k01,k23]
                                ps = T(pp12, [128, NT], "pmain", bufs=4)
                                for cc in range(8):
                                    nc.tensor.matmul(
                                        ps,
                                        wqkv_sb[:, cc, mc * 128 : (mc + 1) * 128],
                                        xc[:, cc, :],
                                        start=(cc == 0),
                                        stop=(cc == 7),
                                    )
                                bias_ap = bqkv_sb[:, mc : mc + 1]
                                if mc < 2:
                                    norm_chunk(pp12, wk, ps, bias_ap,
                                               qknw_sb[:, 0:1], qT[:, mc, nsl])
                                else:
                                    norm_chunk(pp12, wk, ps, bias_ap,
                                               qknw_sb[:, 1:2], kT[:, mc - 2, nsl])
                            for t in range(4):
                                v_proj(pp12, xc, t, wvx_sb, bvx_sb, nt * 4 + t)

                # ---- phase 3+4: attention + output projection, per query tile
                with (
                    tc.tile_pool(name="pa", bufs=2, space="PSUM") as pa,
                    tc.tile_pool(name="atp", bufs=3) as atp,
                    tc.tile_pool(name="asp", bufs=3) as asp,
                    tc.tile_pool(name="outp", bufs=2) as outp,
                    tc.tile_pool(name="osp", bufs=2) as osp,
                ):
                    for nt in range(N // NT):
                        nsl = slice(nt * NT, (nt + 1) * NT)
                        ot = T(outp, [64, HPC, NT], "ot", dt=R32)
                        for hp in range(2):
                            avs_list = []
                            av_list = [
                                T(pa, [128, NT], "avpo", bufs=2)[:65],
                                T(pa, [128, NT], "avpo", bufs=2)[:65],
                            ]
                            for jg in range(6):
                                j0 = 3 * jg
                                for idx in range(2):
                                    h = 2 * hp + idx
                                    prt = slice(64 * idx, 64 * idx + 64)
                                    tp = (64 * idx, 0)
                                    pl = T(pa, [128, 3 * NT], "big", bufs=2)
                                    rhsQ = qT[prt, hp, nsl]
                                    for u in range(3):
                                        nc.tensor.matmul(
                                            pl[:, u * NT : (u + 1) * NT],
                                            kT[prt, hp,
                                               (j0 + u) * 128 : (j0 + u + 1) * 128],
                                            rhsQ, start=True, stop=True,
                                            tile_position=tp,
                                        )
                                    at = T(atp, [128, 3 * NT], "at", dt=R32)
                                    nc.scalar.activation(
                                        at, pl, AF.Exp, bias=zero_sb[:], scale=SCALE
                                    )
                                    av = av_list[idx]
                                    for u in range(3):
                                        nc.tensor.matmul(
                                            av,
                                            vS[:, j0 + u, 65 * h : 65 * h + 65],
                                            at[:, u * NT : (u + 1) * NT],
                                            start=(j0 + u == 0),
                                            stop=(j0 + u == 17),
                                        )
                            for idx in range(2):
                                h = 2 * hp + idx
                                avs = T(asp, [65, NT], "avs", dt=R32)
                                nc.vector.tensor_copy(avs, av_list[idx])
                                dbc = T(pa, [64, NT], "big", bufs=2)
                                nc.tensor.matmul(
                                    dbc, sel_sb, avs, start=True, stop=True
                                )
                                rbc = T(asp, [64, NT], "rbc")
                                nc.vector.reciprocal(rbc, dbc)
                                nc.vector.tensor_mul(ot[:, h, :], avs[0:64, :], rbc)

                        for oc in range(8):
                            po = T(pa, [128, NT], "avpo", bufs=2)
                            for cc in range(4):
                                nc.tensor.matmul(
                                    po,
                                    wproj_sb[:, cc, oc * 128 : (oc + 1) * 128],
                                    ot[:, cc, :],
                                    start=(cc == 0), stop=(cc == 3),
                                )
                            ob = T(osp, [128, NT], "ob")
                            nc.vector.tensor_copy(ob, po)
                            nc.sync.dma_start(
                                outT.rearrange("(o p) f -> p o f", p=128)[:, oc, nsl],
                                ob,
                            )
    _orig = bacc.get_activation_tables

    def _tables(arch):
        t = _orig(arch)
        return {
            name: (set() if name in ("exp_and_others", "natural_log",
                                     "exp_and_friends") else fns)
            for name, fns in t.items()
        }

    bacc.get_activation_tables = _tables
    try:
        nc.compile()
    finally:
        bacc.get_activation_tables = _orig
    return nc


_PROGRAM = None


def _get_program():
    global _PROGRAM
    if _PROGRAM is None:
        _PROGRAM = build_program()
    return _PROGRAM


def _make_in_maps(x, y, qkv_w, qkv_b, kv_w, kv_b, qn_w, kn_w, proj_w, proj_b):
    f = np.float32
    onesb = np.zeros((128, 2), f)
    onesb[0:64, 0] = 1.0
    onesb[64:128, 1] = 1.0
    ones2 = np.zeros((2, 128), f)
    ones2[0, 0:64] = 1.0
    ones2[1, 64:128] = 1.0
    sel64 = np.zeros((65, 64), f)
    sel64[64, :] = 1.0
    qknw = np.stack([np.tile(qn_w, 2), np.tile(kn_w, 2)], axis=1).astype(f)

    in_maps = []
    for core in range(8):
        b, g = divmod(core, 4)
        qs = slice(g * 256, (g + 1) * 256)
        wqkv = np.concatenate([qkv_w[qs], qkv_w[1024:2048][qs]], axis=0)
        bq = np.concatenate([qkv_b[qs], qkv_b[1024:2048][qs]])
        wkv = kv_w[qs]
        bk = kv_b[qs]
        wvx = qkv_w[2048:3072][qs]
        bvxv = qkv_b[2048:3072][qs]
        wvy = kv_w[1024:2048][qs]
        bvyv = kv_b[1024:2048][qs]
        in_maps.append(
            {
                "xT": np.ascontiguousarray(x[b].T, f),
                "yT": np.ascontiguousarray(y[b].T, f),
                "wqkvT": np.ascontiguousarray(wqkv.T, f),
                "bqkv": np.ascontiguousarray(bq.reshape(4, 128).T, f),
                "wkvT": np.ascontiguousarray(wkv.T, f),
                "bkv": np.ascontiguousarray(bk.reshape(2, 128).T, f),
                "wvxT": np.ascontiguousarray(wvx.T, f),
                "bvx": np.ascontiguousarray(bvxv.reshape(1, 256), f),
                "wvyT": np.ascontiguousarray(wvy.T, f),
                "bvy": np.ascontiguousarray(bvyv.reshape(1, 256), f),
                "ones1r": np.ones((1, 128), f),
                "wprojT": np.ascontiguousarray(proj_w[:, qs].T, f),
                "qknw": qknw,
                "onesb": onesb,
                "ones2": ones2,
                "sel64": sel64,
                "vones": np.ones((128, 18, 1), f),
            }
        )
    return in_maps


def run_cores(inputs, trace=False, **kwargs):
    nc = _get_program()
    in_maps = _make_in_maps(**{k: np.asarray(v, np.float32) for k, v in inputs.items()})
    return run_bass_kernel_spmd(
        nc, in_maps, core_ids=list(range(8)), trace=trace, **kwargs
    )


def kernel(**inputs):
    proj_b = np.asarray(inputs["proj_b"], np.float32)
    res = run_cores(inputs).results
    out = np.zeros((B, N, C), np.float32)
    for core in range(8):
        b = core // 4
        out[b] += res[core]["outT"].T
    out += proj_b[None, None, :]
    return out



# revision 2
# speedup vs baseline: 4.3367x; 4.3367x over previous
"""Trainium2 Bass kernel for nn_CrossAttention (B=2, N=2048, M=256, C=1024, H=16).

Sharding: 8 cores = 2 batches x 4 head-groups (4 heads each).
v2: attention restructured around PE array tiling:
 - QK: 2-head row-tiled pairs (as baseline)
 - softmax exp -> bf16 `at`, 2-chunk batches [128, 1024]
 - AV: 4-way column-tiled (32-wide) -> av01/av23 psum stacked [128, NT]
 - denominators: 4-head packed P=1 col-tiled matmul slot -> one psum bank
 - normalize via reciprocal + selB broadcast matmul
 - out-proj: K=64 row-position chains over stacked [128] ot tiles
Host sums the 4 partials per batch and adds proj_b.
"""

import sys

sys.path.insert(0, "/opt/trn_rl_repo")

import numpy as np  # noqa: E402

import concourse.bass as bass  # noqa: E402
import concourse.tile as tile  # noqa: E402
from concourse import bacc, mybir  # noqa: E402
from concourse.bass_utils import run_bass_kernel_spmd  # noqa: E402

F32 = mybir.dt.float32
R32 = mybir.dt.float32r
BF16 = mybir.dt.bfloat16
AF = mybir.ActivationFunctionType
MUL = mybir.AluOpType.mult

H = 16
B = 2
N = 2048          # image tokens
M = 256           # text tokens
C = 1024
HD = 64           # head dim
EPS = 1e-6
S = N + M         # 2304 kv length
HPC = 4           # heads per core
NT = 512          # query tile
SCALE = HD ** -0.5
NCH = 18          # S // 128 kv chunks

_TCNT = [0]


def T(pool, shape, tag, bufs=None, dt=F32):
    _TCNT[0] += 1
    kw = dict(tag=tag, name=f"{tag}_{_TCNT[0]}")
    if bufs is not None:
        kw["bufs"] = bufs
    return pool.tile(shape, dt, **kw)


def build_program(loop_iters=None):
    nc = bacc.Bacc("TRN2", target_bir_lowering=False, debug=False)

    xT = nc.dram_tensor("xT", [C, N], R32, kind="ExternalInput").ap()
    yT = nc.dram_tensor("yT", [C, M], R32, kind="ExternalInput").ap()
    wqkvT = nc.dram_tensor("wqkvT", [C, 2 * HPC * HD], R32, kind="ExternalInput").ap()
    bqkv = nc.dram_tensor("bqkv", [128, 4], F32, kind="ExternalInput").ap()
    wkvT = nc.dram_tensor("wkvT", [C, HPC * HD], R32, kind="ExternalInput").ap()
    wvxT = nc.dram_tensor("wvxT", [C, HPC * HD], R32, kind="ExternalInput").ap()
    wvyT = nc.dram_tensor("wvyT", [C, HPC * HD], R32, kind="ExternalInput").ap()
    bvx = nc.dram_tensor("bvx", [1, HPC * HD], R32, kind="ExternalInput").ap()
    bvy = nc.dram_tensor("bvy", [1, HPC * HD], R32, kind="ExternalInput").ap()
    ones1r = nc.dram_tensor("ones1r", [1, 128], R32, kind="ExternalInput").ap()
    bkv = nc.dram_tensor("bkv", [128, 2], F32, kind="ExternalInput").ap()
    wproj2 = nc.dram_tensor("wproj2", [128, 2 * C], BF16, kind="ExternalInput").ap()
    qknw = nc.dram_tensor("qknw", [128, 2], F32, kind="ExternalInput").ap()
    onesb = nc.dram_tensor("onesb", [128, 2], R32, kind="ExternalInput").ap()
    ones2 = nc.dram_tensor("ones2", [2, 128], R32, kind="ExternalInput").ap()
    onesd = nc.dram_tensor("onesd", [128, 4], BF16, kind="ExternalInput").ap()
    selB = nc.dram_tensor("selB", [128, 2 * 128], R32, kind="ExternalInput").ap()
    outT = nc.dram_tensor("outT", [C, N], F32, kind="ExternalOutput").ap()

    with tile.TileContext(nc) as tc:
        with (
            tc.tile_pool(name="const", bufs=1) as const,
            tc.tile_pool(name="sing", bufs=1) as sing,
        ):
            yT_sb = T(const, [128, 8, M], "yT", dt=R32)
            nc.sync.dma_start(yT_sb, yT.rearrange("(o p) f -> p o f", p=128))
            wkv_sb = T(const, [128, 8, HPC * HD], "wkv", dt=R32)
            nc.sync.dma_start(wkv_sb, wkvT.rearrange("(o p) f -> p o f", p=128))
            wvy_sb = T(const, [128, 8, HPC * HD], "wvy", dt=R32)
            nc.sync.dma_start(wvy_sb, wvyT.rearrange("(o p) f -> p o f", p=128))
            wvx_sb = T(const, [128, 8, HPC * HD], "wvx", dt=R32)
            nc.sync.dma_start(wvx_sb, wvxT.rearrange("(o p) f -> p o f", p=128))
            bvx_sb = T(const, [1, HPC * HD], "bvx", dt=R32)
            nc.sync.dma_start(bvx_sb, bvx)
            bvy_sb = T(const, [1, HPC * HD], "bvy", dt=R32)
            nc.sync.dma_start(bvy_sb, bvy)
            ones1_sb = T(const, [1, 128], "ones1r", dt=R32)
            nc.sync.dma_start(ones1_sb, ones1r)
            wqkv_sb = T(const, [128, 8, 2 * HPC * HD], "wqkv", dt=R32)
            wqkv_r = wqkvT.rearrange("(o p) f -> p o f", p=128)
            for cc in range(8):
                nc.sync.dma_start(wqkv_sb[:, cc], wqkv_r[:, cc])
            wproj_sb = T(const, [128, 2, C], "wproj", dt=BF16)
            nc.sync.dma_start(wproj_sb, wproj2.rearrange("p (a o) -> p a o", a=2))
            bqkv_sb = T(const, [128, 4], "bqkv")
            nc.sync.dma_start(bqkv_sb, bqkv)
            bkv_sb = T(const, [128, 2], "bkv")
            nc.sync.dma_start(bkv_sb, bkv)
            qknw_sb = T(const, [128, 2], "qknw")
            nc.sync.dma_start(qknw_sb, qknw)
            onesb_sb = T(const, [128, 2], "onesb", dt=R32)
            nc.sync.dma_start(onesb_sb, onesb)
            ones2_sb = T(const, [2, 128], "ones2", dt=R32)
            nc.sync.dma_start(ones2_sb, ones2)
            onesd_sb = T(const, [128, 4], "onesd", dt=BF16)
            nc.sync.dma_start(onesd_sb, onesd)
            selB_sb = T(const, [128, 2, 128], "selB", dt=R32)
            nc.sync.dma_start(selB_sb, selB.rearrange("p (a o) -> p a o", a=2))
            eps_sb = T(const, [128, 1], "epsc")
            nc.vector.memset(eps_sb, float(EPS))
            zero_sb = T(const, [128, 1], "zeroc")
            nc.vector.memset(zero_sb, 0.0)

            # persistent activations: channel-on-partition layouts
            qT = T(sing, [128, 2, N], "qT", dt=R32)       # [2 heads x 64d, hp, n]
            kT = T(sing, [128, 2, S], "kT", dt=R32)
            vS = T(sing, [128, NCH, HPC * HD], "vS", dt=BF16)  # [s%128, s//128, h*64+d]
            ds = T(sing, [128, NT], "ds", dt=R32)         # reciprocal denominators

            def norm_chunk(pool_ps, pool_wk, psum, bias_ap, w_col, dest):
                """dest = (psum + bias) * rsqrt(mean_d((psum+bias)^2)+eps) * w"""
                nsz = psum.shape[-1]
                tb = T(pool_wk, [128, NT], "w")[:, :nsz]
                nc.vector.tensor_scalar_add(tb, psum, bias_ap)
                sq = T(pool_wk, [128, NT], "w", dt=R32)[:, :nsz]
                nc.vector.tensor_mul(sq, tb, tb)
                ssp = T(pool_ps, [2, NT], "paux", bufs=3)[:, :nsz]
                nc.tensor.matmul(ssp, onesb_sb, sq, start=True, stop=True)
                lnv = T(pool_wk, [2, NT], "w2", bufs=8)[:, :nsz]
                nc.scalar.activation(
                    lnv, ssp, AF.Ln, bias=eps_sb[0:2], scale=1.0 / HD
                )
                rmsv = T(pool_wk, [2, NT], "w2", bufs=8, dt=R32)[:, :nsz]
                nc.scalar.activation(rmsv, lnv, AF.Exp, bias=zero_sb[0:2], scale=-0.5)
                rbc = T(pool_ps, [128, NT], "paux", bufs=3)[:, :nsz]
                nc.tensor.matmul(rbc, ones2_sb, rmsv, start=True, stop=True)
                t2 = T(pool_wk, [128, NT], "w")[:, :nsz]
                nc.vector.tensor_mul(t2, tb, rbc)
                nc.vector.tensor_scalar_mul(dest, t2, w_col)

            def v_proj(pool_ps, src_sb, t, w_sb, b_sb, j):
                """vS[:, j] = (src.T @ wv + bv) directly in [s, d] layout."""
                pv = T(pool_ps, [128, HPC * HD], "pmain", bufs=4)
                for cc in range(8):
                    nc.tensor.matmul(
                        pv,
                        src_sb[:, cc, t * 128 : (t + 1) * 128],
                        w_sb[:, cc, :],
                        start=(cc == 0),
                        stop=False,
                    )
                nc.tensor.matmul(pv, ones1_sb, b_sb, start=False, stop=True)
                nc.vector.tensor_copy(out=vS[:, j, :], in_=pv)

            # ---- phase 1: KV projection of y (text tokens -> kv rows 2048..2303)
            import contextlib
            with contextlib.ExitStack() as _les:
                if loop_iters is not None:
                    _les.enter_context(tc.For_i(0, loop_iters, 1))
                with (
                    tc.tile_pool(name="pp12", bufs=3, space="PSUM") as pp12,
                    tc.tile_pool(name="wk", bufs=12) as wk,
                ):
                    for mc in range(2):  # [k01, k23]
                        ps = T(pp12, [128, NT], "pmain", bufs=4)[:, :M]
                        for cc in range(8):
                            nc.tensor.matmul(
                                ps,
                                wkv_sb[:, cc, mc * 128 : (mc + 1) * 128],
                                yT_sb[:, cc, :],
                                start=(cc == 0),
                                stop=(cc == 7),
                            )
                        norm_chunk(
                            pp12, wk, ps, bkv_sb[:, mc : mc + 1],
                            qknw_sb[:, 1:2], kT[:, mc, N : N + M],
                        )
                    for t in range(2):
                        v_proj(pp12, yT_sb, t, wvy_sb, bvy_sb, 16 + t)

                    # ---- phase 2: QKV projection of x
                    with tc.tile_pool(name="xin", bufs=2) as xin:
                        for nt in range(N // NT):
                            nsl = slice(nt * NT, (nt + 1) * NT)
                            xc = T(xin, [128, 8, NT], "xc", dt=R32)
                            nc.sync.dma_start(
                                xc, xT.rearrange("(o p) f -> p o f", p=128)[:, :, nsl]
                            )
                            for mc in range(4):  # [q01,q23,k01,k23]
                                ps = T(pp12, [128, NT], "pmain", bufs=4)
                                for cc in range(8):
                                    nc.tensor.matmul(
                                        ps,
                                        wqkv_sb[:, cc, mc * 128 : (mc + 1) * 128],
                                        xc[:, cc, :],
                                        start=(cc == 0),
                                        stop=(cc == 7),
                                    )
                                bias_ap = bqkv_sb[:, mc : mc + 1]
                                if mc < 2:
                                    norm_chunk(pp12, wk, ps, bias_ap,
                                               qknw_sb[:, 0:1], qT[:, mc, nsl])
                                else:
                                    norm_chunk(pp12, wk, ps, bias_ap,
                                               qknw_sb[:, 1:2], kT[:, mc - 2, nsl])
                            for t in range(4):
                                v_proj(pp12, xc, t, wvx_sb, bvx_sb, nt * 4 + t)

                # ---- phase 3+4: attention + output projection, per query tile
                with (
                    tc.tile_pool(name="pbig", bufs=1, space="PSUM") as pbig,
                    tc.tile_pool(name="pacc", bufs=1, space="PSUM") as pacc,
                    tc.tile_pool(name="atp", bufs=1) as atp,
                    tc.tile_pool(name="asp", bufs=1) as asp,
                    tc.tile_pool(name="osp", bufs=2) as osp,
                ):
                    for nt in range(N // NT):
                        nsl = slice(nt * NT, (nt + 1) * NT)
                        av01 = T(pacc, [128, NT], "av01", bufs=1)
                        av23 = T(pacc, [128, NT], "av23", bufs=1)
                        dn = T(pacc, [128, NT], "dn", bufs=1)
                        # junk partitions read 1.0 through the bc matmul;
                        # the denom chains' start=True re-zero their own rows
                        nc.vector.memset(dn, 1.0)
                        at_l = [[None, None] for _ in range(HPC)]
                        for jg in range(9):  # 2-chunk batches
                            ats = []
                            for hp in range(2):
                                pl = [None, None]
                                for idx in range(2):
                                    pl[idx] = T(pbig, [128, 2 * NT], "big", bufs=2)
                                for u in range(2):
                                    j = 2 * jg + u
                                    for idx in range(2):
                                        prt = slice(64 * idx, 64 * idx + 64)
                                        nc.tensor.matmul(
                                            pl[idx][:, u * NT : (u + 1) * NT],
                                            kT[prt, hp, j * 128 : (j + 1) * 128],
                                            qT[prt, hp, nsl],
                                            start=True, stop=True,
                                            tile_position=(64 * idx, 0),
                                        )
                                for idx in range(2):
                                    at = T(atp, [128, 2 * NT], "at", bufs=6, dt=BF16)
                                    nc.scalar.activation(
                                        at, pl[idx], AF.Exp,
                                        bias=zero_sb[:], scale=SCALE,
                                    )
                                    ats.append(at)
                            # AV: 4-way col-tiled, heads stacked per pair
                            for u in range(2):
                                j = 2 * jg + u
                                usl = slice(u * NT, (u + 1) * NT)
                                for hp in range(2):
                                    avp = av01 if hp == 0 else av23
                                    for q in range(2):
                                        h2 = 2 * hp + q
                                        csl = slice(h2 * 64, h2 * 64 + 64)
                                        nc.tensor.matmul(
                                            avp[64 * q : 64 * (q + 1), :],
                                            vS[:, j, csl],
                                            ats[h2][:, usl],
                                            start=(j == 0), stop=(j == NCH - 1),
                                            tile_position=(0, 64 * q),
                                            skip_group_check=True,
                                        )
                                # denominators: 4 heads, P=1 each
                                if hp == 1:
                                    for q in range(4):
                                        nc.tensor.matmul(
                                            dn[32 * q : 32 * q + 1, :],
                                            onesd_sb[:, q : q + 1],
                                            ats[q][:, usl],
                                            start=(j == 0), stop=(j == NCH - 1),
                                            tile_position=(0, 32 * q),
                                            skip_group_check=True,
                                        )
                        # normalize: rinv broadcast then multiply
                        with nc.allow_low_precision(
                            reason="float32r output is fp32 storage"
                        ):
                            nc.vector.reciprocal(ds, dn)
                        ot_l = []
                        for hp in range(2):
                            bc = T(pbig, [128, NT], "po", bufs=1)
                            nc.tensor.matmul(
                                bc, selB_sb[:, hp, :], ds,
                                start=True, stop=True,
                            )
                            avp = av01 if hp == 0 else av23
                            avc = T(asp, [128, NT], "avc", bufs=2, dt=BF16)
                            nc.vector.tensor_copy(out=avc, in_=avp)
                            ot = T(asp, [128, NT], "ot", bufs=2, dt=BF16)
                            nc.vector.tensor_mul(ot, avc, bc)
                            ot_l.append(ot)

                        # out-projection: 2 K=128 chunks (head pairs) per oc
                        for oc in range(8):
                            po = T(pbig, [128, NT], "po", bufs=1)
                            for p in range(2):
                                nc.tensor.matmul(
                                    po,
                                    wproj_sb[:, p, oc * 128 : (oc + 1) * 128],
                                    ot_l[p],
                                    start=(p == 0), stop=(p == 1),
                                )
                            ob = T(osp, [128, NT], "ob")
                            nc.vector.tensor_copy(ob, po)
                            nc.sync.dma_start(
                                outT.rearrange("(o p) f -> p o f", p=128)[:, oc, nsl],
                                ob,
                            )
    _orig = bacc.get_activation_tables

    def _tables(arch):
        t = _orig(arch)
        return {
            name: (set() if name in ("exp_and_others", "natural_log",
                                     "exp_and_friends") else fns)
            for name, fns in t.items()
        }

    bacc.get_activation_tables = _tables
    try:
        nc.compile()
    finally:
        bacc.get_activation_tables = _orig
    return nc


_PROGRAM = None


def _get_program():
    global _PROGRAM
    if _PROGRAM is None:
        _PROGRAM = build_program()
    return _PROGRAM


def _make_in_maps(x, y, qkv_w, qkv_b, kv_w, kv_b, qn_w, kn_w, proj_w, proj_b):
    import ml_dtypes

    f = np.float32
    bf = ml_dtypes.bfloat16
    onesb = np.zeros((128, 2), f)
    onesb[0:64, 0] = 1.0
    onesb[64:128, 1] = 1.0
    ones2 = np.zeros((2, 128), f)
    ones2[0, 0:64] = 1.0
    ones2[1, 64:128] = 1.0
    onesd = np.ones((128, 4), bf)
    selB = np.zeros((128, 2, 128), f)
    for hp in range(2):
        selB[32 * (2 * hp), hp, 0:64] = 1.0
        selB[32 * (2 * hp + 1), hp, 64:128] = 1.0
    selB = selB.reshape(128, 256)
    qknw = np.stack([np.tile(qn_w, 2), np.tile(kn_w, 2)], axis=1).astype(f)

    in_maps = []
    for core in range(8):
        b, g = divmod(core, 4)
        qs = slice(g * 256, (g + 1) * 256)
        wqkv = np.concatenate([qkv_w[qs], qkv_w[1024:2048][qs]], axis=0)
        bq = np.concatenate([qkv_b[qs], qkv_b[1024:2048][qs]])
        wkv = kv_w[qs]
        bk = kv_b[qs]
        wvx = qkv_w[2048:3072][qs]
        bvxv = qkv_b[2048:3072][qs]
        wvy = kv_w[1024:2048][qs]
        bvyv = kv_b[1024:2048][qs]
        wp = np.ascontiguousarray(proj_w[:, qs].T, f)  # [256, 1024]
        wproj2 = np.ascontiguousarray(
            wp.reshape(2, 128, C).transpose(1, 0, 2).reshape(128, 2 * C)
        ).astype(bf)
        in_maps.append(
            {
                "xT": np.ascontiguousarray(x[b].T, f),
                "yT": np.ascontiguousarray(y[b].T, f),
                "wqkvT": np.ascontiguousarray(wqkv.T, f),
                "bqkv": np.ascontiguousarray(bq.reshape(4, 128).T, f),
                "wkvT": np.ascontiguousarray(wkv.T, f),
                "bkv": np.ascontiguousarray(bk.reshape(2, 128).T, f),
                "wvxT": np.ascontiguousarray(wvx.T, f),
                "bvx": np.ascontiguousarray(bvxv.reshape(1, 256), f),
                "wvyT": np.ascontiguousarray(wvy.T, f),
                "bvy": np.ascontiguousarray(bvyv.reshape(1, 256), f),
                "ones1r": np.ones((1, 128), f),
                "wproj2": wproj2,
                "qknw": qknw,
                "onesb": onesb,
                "ones2": ones2,
                "onesd": onesd,
                "selB": selB,
            }
        )
    return in_maps


def run_cores(inputs, trace=False, **kwargs):
    nc = _get_program()
    in_maps = _make_in_maps(**{k: np.asarray(v, np.float32) for k, v in inputs.items()})
    return run_bass_kernel_spmd(
        nc, in_maps, core_ids=list(range(8)), trace=trace, **kwargs
    )


def kernel(**inputs):
    proj_b = np.asarray(inputs["proj_b"], np.float32)
    res = run_cores(inputs).results
    out = np.zeros((B, N, C), np.float32)
    for core in range(8):
        b = core // 4
        out[b] += res[core]["outT"].T
    out += proj_b[None, None, :]
    return out
